# revision 25
# baseline (speedup 1.0000x reference)
"""Trainium2 Bass kernel for nn_ImitationHead (dense_mlp).

Computation (per batch row b of 256):
  h  = mean(z[b], spatial)                # [512] <- z [512,16,16]
  h  = relu-MLP chain 512->512->256->128->64
  goal = [goal_point[b,0,3], goal_point[b,1,3], goal_point_speed[b]]
  GRU (hidden 64, input [x(3); goal(3)]) unrolled 8 steps, each step
  followed by an output MLP 64->4(relu)->4->3 producing dx; x += dx.
  Output: the 8 x values -> [256, 8, 3].

Sharding: pure data parallel, batch 256 -> 8 cores x 32.

Design (v3):
  - z shard viewed as [16384, 256]; 16 DMAs of [128p, 2, 4, 256] (1 MiB)
    on the SP HWDGE ring, saturating the ~358 GB/s per-core HBM limit.
    The channel permutation (chunk j, partition p <-> channel 4p+j) is
    undone by permuting the rows of the layer-1 weight on the host.
  - free-axis reduce -> hT[channel_p, batch]: DVE handles j=0..2 (one
    [128,2,256] reduce per DMA+j, bf16 out), ACT handles j=3 via the
    activation accum_out path (f32 + one bf16 convert at the end), so
    the post-stream reduce tail is ~1.9us and ACT/DVE split the load.
  - constants: a small "early pack" (goal matmul weights, goal values,
    MLP biases) loads on the ACT HWDGE ring at t=0; the big bf16 MLP
    weights + the f32 GRU pack load on the SP ring AFTER the z DMAs,
    landing exactly when each consumer needs them (w1 -> L1, wrest ->
    L2..L4, grupack -> GRU init).  No serial constant phase.
  - join MLP in bf16 (fp32 PSUM); ReLU+bias fused on DVE via
    tensor_scalar (add bias, max 0) -- cheaper chain hops than ACT.
  - GRU in fp32, persistent PSUM accumulators (prz/pin/phn/pd1):
    * z-gate preactivation accumulated NEGATED -> sigmoid emits
      zc = 1-z directly.
    * state delta dlt = zc*(hh-n) split as a = -zc*hh (computed on
      GPSIMD while ACT runs tanh) and b = zc*n (on-chain);
      every "-W@dlt" PSUM update becomes "+W@a + W@b" using the SAME
      +W.T lhsT (no negated weight copies needed).
    * hh' = hh + a + b on GPSIMD, off the critical chain.
    * phn/pin are mirrored into SBUF (phnS by ACT, pinS by DVE) right
      after their last per-step PSUM update, so the chain ops
      r*phn + pin run at SBUF speed.
    * x-recurrence folds through the output MLP: gi_x +=
      (W_ihx @ W23.T) @ relu(pd1)  (wixo matmuls).
  - biases fold in as an all-ones input row; the 4->4 and 4->3 output
    layers fold into one 4->3 matrix on the host; the mean's 1/256
    folds into the (bf16) layer-1 weights on the host.
"""

import numpy as np
import ml_dtypes
from contextlib import ExitStack

N_CORES = 8
B = 256
B_SH = B // N_CORES       # 32 batch rows per core
C = 512                   # channels
S = 256                   # spatial 16*16
HID = 64
T = 8                     # pred_len
ROWS = B_SH * C           # 16384 z rows per core
N_DMA = 16                # z DMAs per core
H_PER = 2                 # batch blocks per z DMA
J = 4                     # 256-chunks per partition per batch block

# early pack (f32): wgobt(192) | goalones(32) | biases(8)
EP_COLS = 192 + B_SH + 8
# gru pack (f32): whhbt(192) | ow1bt(4) | wixobt(192) | ow23bt(3)
#                 | whhnbt(192) | ow1nbt(4)
GP_COLS = 192 + 4 + 192 + 3 + 192 + 4

_CACHE: dict = {}


def _build_program():
    import concourse.bacc as bacc
    import concourse.tile as tile
    from concourse import mybir

    f32 = mybir.dt.float32
    bf16 = mybir.dt.bfloat16
    AF = mybir.ActivationFunctionType
    AX = mybir.AxisListType
    ALU = mybir.AluOpType

    nc = bacc.Bacc("TRN2", target_bir_lowering=False, debug=False)

    z = nc.dram_tensor("z", [ROWS, S], f32, kind="ExternalInput")
    w1_d = nc.dram_tensor("w1p", [128, 4 * 512], bf16, kind="ExternalInput")
    wrest_d = nc.dram_tensor("wrest", [128, 4 * 256 + 2 * 128 + 64], bf16,
                             kind="ExternalInput")
    ep_d = nc.dram_tensor("epack", [128, EP_COLS], f32, kind="ExternalInput")
    gp_d = nc.dram_tensor("gpack", [128, GP_COLS], f32, kind="ExternalInput")
    out_d = nc.dram_tensor("out", [3 * T, B_SH], f32, kind="ExternalOutput")

    with tile.TileContext(nc) as tc, ExitStack() as ctx:
        consts = ctx.enter_context(tc.tile_pool(name="consts", bufs=1))
        zpool = ctx.enter_context(tc.tile_pool(name="zpool", bufs=3))
        hpool = ctx.enter_context(tc.tile_pool(name="hpool", bufs=1))
        work = ctx.enter_context(tc.tile_pool(name="work", bufs=2))
        xpool = ctx.enter_context(tc.tile_pool(name="xpool", bufs=2))
        psum_mlp = ctx.enter_context(
            tc.tile_pool(name="psum_mlp", bufs=2, space="PSUM"))
        psum_gru = ctx.enter_context(
            tc.tile_pool(name="psum_gru", bufs=1, space="PSUM"))

        # early pack on the ACT HWDGE ring, issued first
        ep = consts.tile([128, EP_COLS], f32)
        nc.scalar.dma_start(out=ep, in_=ep_d[:])
        wgo = ep[0:4, 0:192]
        gl = ep[0:4, 192:192 + B_SH]
        bs = ep[0:128, 192 + B_SH:EP_COLS]

        # --- z stream on the SP ring: 15 x 1MiB (2 batches) + 2 x 0.5MiB
        # (1 batch) so the post-stream reduce tail is one batch's worth.
        # Row d*1024 + h*512 + 4p + j -> batch b = 2d+h, channel 4p+j.
        hTs = [hpool.tile([128, B_SH], bf16, tag=f"hT{j}", name=f"hT{j}")
               for j in range(4)]
        hT3 = hTs[3]
        junk = hpool.tile([128, S], f32)         # ACT accum main out
        z_r = z[:].rearrange("(b p j) s -> b p j s", p=128, j=J)  # per batch
        # batches 0..29: 15 x 1MiB DMAs; DVE reduces j=0..2, ACT j=3.
        # batch 30: one 0.5MiB DMA, same split.  batch 31: four 128KiB
        # j-slice DMAs, all reduced on DVE as they land, so the
        # post-stream reduce tail is minimal.
        for d in range(N_DMA):
            bats = [2 * d, 2 * d + 1] if d < N_DMA - 1 else [30]
            nb = len(bats)
            zt = zpool.tile([128, nb, J, S], f32, tag=f"zt{nb}")
            nc.sync.dma_start(
                out=zt, in_=z_r[bats[0]:bats[0] + nb].rearrange(
                    "b p j s -> p b j s"))
            for j in range(3):
                with nc.allow_low_precision(reason="bf16 matmul rhs"):
                    nc.vector.tensor_reduce(
                        out=hTs[j][:, bats[0]:bats[0] + nb],
                        in_=zt[:, :, j, :], axis=AX.X, op=ALU.add)
            for h in range(nb):
                b = bats[h]
                with nc.allow_low_precision(reason="bf16 matmul rhs"):
                    nc.scalar.activation(
                        out=junk, in_=zt[:, h, 3, :], func=AF.Copy,
                        accum_out=hT3[:, b:b + 1])
        for j in range(J):
            ztj = zpool.tile([128, S], f32, tag=f"ztj{j}")
            nc.sync.dma_start(out=ztj, in_=z_r[31, :, j, :])
            with nc.allow_low_precision(reason="bf16 matmul rhs"):
                nc.vector.tensor_reduce(
                    out=hTs[j][:, 31:32], in_=ztj,
                    axis=AX.X, op=ALU.add)

        # big constants on the SP ring, queued AFTER the z stream, split
        # into small DMAs so each lands (plus the 900ns DMA-sem delay)
        # just before its consumer needs it: w1 halves -> L1, w2 -> L2,
        # w34 -> L3/L4, grupack -> GRU init matmuls.
        w1 = consts.tile([128, 4 * 512], bf16)
        nc.sync.dma_start(out=w1[:, 0:1024], in_=w1_d[:, 0:1024])
        nc.sync.dma_start(out=w1[:, 1024:2048], in_=w1_d[:, 1024:2048])
        wrest = consts.tile([128, 4 * 256 + 2 * 128 + 64], bf16)
        nc.sync.dma_start(out=wrest[:, 0:1024], in_=wrest_d[:, 0:1024])
        nc.sync.dma_start(out=wrest[:, 1024:1344], in_=wrest_d[:, 1024:1344])
        gp = consts.tile([128, GP_COLS], f32)
        nc.sync.dma_start(out=gp, in_=gp_d[:])
        whh = gp[0:65, 0:192]
        ow1 = gp[0:65, 192:196]
        wixo = gp[0:33, 196:388]
        ow23 = gp[0:33, 388:391]
        whhn = gp[0:64, 391:583]
        ow1n = gp[0:64, 583:587]

        # --- GRU persistent PSUM accumulators; goal-part inits run
        # during the stream (they only need the early pack).
        prz = psum_gru.tile([128, B_SH], f32, tag="prz")   # [r; zc] pre-act
        pin = psum_gru.tile([64, B_SH], f32, tag="pin")    # i_n pre-act
        phn = psum_gru.tile([64, B_SH], f32, tag="phn")    # h_n pre-act
        pd1 = psum_gru.tile([4, B_SH], f32, tag="pd1")     # oW1@hh+ob1
        kw = dict(skip_group_check=True)
        nc.tensor.matmul(prz, wgo[:, 0:128], gl, start=True, stop=False, **kw)
        nc.tensor.matmul(pin, wgo[:, 128:192], gl, start=True, stop=False, **kw)
        pinS = hpool.tile([64, B_SH], f32, tag="pinS")
        nc.vector.tensor_copy(pinS, pin)                   # during stream
        # gate order in prz is [zc(0:64); r(64:128)] so every two-input
        # elementwise op sees equal base partitions (BIR constraint);
        # the phn mirror lives at partitions 64:128 to pair with r.
        phnS_t = hpool.tile([128, B_SH], f32, tag="phnS")
        phnS = phnS_t[64:128, :]

        # ACT table warmup (sigmoid_and_others covers sigmoid/tanh/copy).
        warm = consts.tile([1, 1], f32)
        nc.vector.memset(warm, 0.0)
        nc.scalar.activation(warm, warm, AF.Sigmoid)
        nc.scalar.activation(warm, warm, AF.Tanh)

        # d1g: relu(pd1) with ones row at partition 32; rows 4:32 stay 0.
        d1g = hpool.tile([33, B_SH], f32)
        nc.vector.memset(d1g[0:33, :], 0.0)
        nc.vector.memset(d1g[32:33, :], 1.0)
        # hhg rows 0:64 = GRU hidden state (in-place across steps), row 64 = 1.
        hhg = hpool.tile([65, B_SH], f32)
        nc.vector.memset(hhg[64:65, :], 1.0)

        def relu_bias(out_ap, pt, bias_ap, eng="dve"):
            # out = max(pt + bias, 0); split across DVE and ACT so the
            # per-layer relu phase runs in parallel on both engines.
            with nc.allow_low_precision(reason="bf16 mlp activations"):
                if eng == "dve":
                    nc.vector.tensor_scalar(
                        out=out_ap, in0=pt, scalar1=bias_ap, scalar2=0.0,
                        op0=ALU.add, op1=ALU.max)
                else:
                    nc.scalar.activation(out_ap, pt, AF.Relu, bias=bias_ap)

        # --- join MLP (transposed, bf16): hN_T = relu(W @ h_T + b) ---
        h1 = hpool.tile([128, 4, B_SH], bf16)
        for m in range(4):
            pt = psum_mlp.tile([128, B_SH], f32, tag="mlp")
            for k in range(4):
                nc.tensor.matmul(
                    pt, w1[:, k * 512 + m * 128:k * 512 + (m + 1) * 128],
                    hTs[k], start=(k == 0), stop=(k == 3))
            relu_bias(h1[:, m, :], pt, bs[:, m:m + 1],
                      eng=("act" if m < 2 else "dve"))
        h2 = hpool.tile([128, 2, B_SH], bf16)
        for m in range(2):
            pt = psum_mlp.tile([128, B_SH], f32, tag="mlp")
            for k in range(4):
                nc.tensor.matmul(
                    pt, wrest[:, k * 256 + m * 128:k * 256 + (m + 1) * 128],
                    h1[:, k, :], start=(k == 0), stop=(k == 3))
            relu_bias(h2[:, m, :], pt, bs[:, 4 + m:5 + m],
                      eng=("act" if m == 0 else "dve"))
        h3 = hpool.tile([128, B_SH], bf16)
        pt = psum_mlp.tile([128, B_SH], f32, tag="mlp")
        for k in range(2):
            nc.tensor.matmul(pt, wrest[:, 1024 + k * 128:1024 + (k + 1) * 128],
                             h2[:, k, :], start=(k == 0), stop=(k == 1))
        relu_bias(h3, pt, bs[:, 6:7])
        pt = psum_mlp.tile([64, B_SH], f32, tag="mlp")
        nc.tensor.matmul(pt, wrest[:, 1280:1344], h3, start=True, stop=True)
        relu_bias(hhg[0:64, :], pt, bs[0:64, 7:8])

        # --- hidden-state-dependent GRU init matmuls + SBUF mirrors
        nc.tensor.matmul(prz, whh[:, 0:128], hhg, start=False, stop=False, **kw)
        nc.tensor.matmul(phn, whh[:, 128:192], hhg, start=True, stop=False, **kw)
        nc.tensor.matmul(pd1, ow1, hhg, start=True, stop=False, **kw)
        # init mirror on DVE so the ACT queue goes straight to sigmoid(t=0)
        nc.vector.tensor_copy(phnS, phn)

        # --- GRU: 8 unrolled steps ---
        x_prev = None
        for t in range(T):
            last = t == T - 1
            rz = work.tile([128, B_SH], f32, tag="rz")
            nc.scalar.activation(rz, prz, AF.Sigmoid)       # [zc; r]
            tmp = work.tile([64, B_SH], f32, tag="tmp")
            nc.vector.tensor_mul(tmp, rz[64:128, :], phnS)  # r * h_n
            ptm = work.tile([64, B_SH], f32, tag="ptm")
            nc.vector.tensor_add(ptm, tmp, pinS)            # + i_n
            # a = zc*hh on GPSIMD, overlapping the tanh; the A-matmuls
            # use the negated lhsT copies (whhn/ow1n).
            a_t = work.tile([64, B_SH], f32, tag="a_t")
            nc.gpsimd.tensor_mul(a_t, rz[0:64, :], hhg[0:64, :])
            nc.tensor.matmul(pd1, ow1n, a_t,
                             start=False, stop=False, **kw)
            if not last:
                nc.tensor.matmul(prz, whhn[:, 0:128], a_t,
                                 start=False, stop=False, **kw)
                nc.tensor.matmul(phn, whhn[:, 128:192], a_t,
                                 start=False, stop=False, **kw)
            n_t = work.tile([64, B_SH], f32, tag="n_t")
            nc.scalar.activation(n_t, ptm, AF.Tanh)
            b_t = work.tile([64, B_SH], f32, tag="b_t")
            nc.vector.tensor_mul(b_t, rz[0:64, :], n_t)     # b = zc*n
            # B-matmuls: pd1 first (it gates the d1g -> wixo chain)
            nc.tensor.matmul(pd1, ow1[0:64, :], b_t,
                             start=False, stop=last, **kw)
            if not last:
                nc.tensor.matmul(prz, whh[0:64, 0:128], b_t,
                                 start=False, stop=False, **kw)
                nc.tensor.matmul(phn, whh[0:64, 128:192], b_t,
                                 start=False, stop=(t == T - 2), **kw)
            nc.vector.tensor_scalar_max(d1g[0:4, :], pd1, 0.0)  # d1(hh')
            if not last:
                # x-recurrence folded through d1g
                nc.tensor.matmul(prz, wixo[:, 0:128], d1g,
                                 start=False, stop=(t == T - 2), **kw)
                nc.tensor.matmul(pin, wixo[:, 128:192], d1g,
                                 start=False, stop=(t == T - 2), **kw)
                # hh' = hh - a + b on GPSIMD (off the critical chain)
                hh1 = work.tile([64, B_SH], f32, tag="hh1")
                nc.gpsimd.tensor_sub(hh1, hhg[0:64, :], a_t)
                nc.gpsimd.tensor_add(hhg[0:64, :], hh1, b_t)
                # refresh the SBUF mirrors for the next step
                nc.vector.tensor_copy(pinS, pin)
                nc.scalar.activation(phnS, phn, AF.Copy)

            # x output (off the critical chain)
            pd3 = psum_gru.tile([3, B_SH], f32, tag="pd3")
            nc.tensor.matmul(pd3, ow23, d1g, start=True, stop=True)
            x_new = xpool.tile([3, B_SH], f32, tag="x")
            if x_prev is None:
                nc.vector.tensor_copy(x_new, pd3)
            else:
                nc.vector.tensor_add(x_new, x_prev, pd3)
            nc.sync.dma_start(out=out_d[3 * t:3 * t + 3, :], in_=x_new)
            x_prev = x_new

    nc.compile()
    return nc


def _get_program():
    if "nc" not in _CACHE:
        _CACHE["nc"] = _build_program()
    return _CACHE["nc"]


def make_in_maps(**inputs) -> list[dict]:
    """Host-side packing + data-parallel sharding -> one in_map per core."""
    f = lambda a: np.ascontiguousarray(np.asarray(a, dtype=np.float32))
    bf = lambda a: np.ascontiguousarray(a.astype(ml_dtypes.bfloat16))
    z = f(inputs["z"]).reshape(B, C, S)
    gp_ = f(inputs["goal_point"])
    gps = f(inputs["goal_point_speed"])
    W_ih, W_hh = f(inputs["W_ih"]), f(inputs["W_hh"])
    b_ih, b_hh = f(inputs["b_ih"]), f(inputs["b_hh"])
    oW1, ob1 = f(inputs["oW1"]), f(inputs["ob1"])
    oW2, ob2 = f(inputs["oW2"]), f(inputs["ob2"])
    oW3, ob3 = f(inputs["oW3"]), f(inputs["ob3"])

    # gate layout on-chip is [z | r | n] (z first so sigmoid's zc output
    # lands at base partition 0); the z block is sign-flipped so sigmoid
    # emits zc = 1-z directly.  Torch order is [r | z | n].
    gperm = np.concatenate([np.arange(64, 128), np.arange(0, 64),
                            np.arange(128, 192)])
    pm = np.ones(192, np.float32)
    pm[0:64] = -1.0
    zrn = lambda a: a[..., gperm] * pm

    # layer-1 weight: fold the 1/S mean scale and the z-layout channel
    # permutation (chunk j, partition p <-> channel 4p+j).
    jw1t = f(inputs["jW1"]).T * np.float32(1.0 / S)
    perm = (4 * np.arange(128)[None, :] + np.arange(4)[:, None]).reshape(-1)
    jw1t = np.ascontiguousarray(jw1t[perm])         # [512, 512]
    # SBUF layout [128p, k-major]: row (k*128+p) -> col k*512+m
    jw1t_sb = bf(jw1t.reshape(4, 128, 512).transpose(1, 0, 2).reshape(128, 2048))
    jw2t = f(inputs["jW2"]).T                       # [512, 256]
    jw2t_sb = jw2t.reshape(4, 128, 256).transpose(1, 0, 2).reshape(128, 1024)
    jw3t = f(inputs["jW3"]).T                       # [256, 128]
    jw3t_sb = jw3t.reshape(2, 128, 128).transpose(1, 0, 2).reshape(128, 256)
    jw4t = f(inputs["jW4"]).T                       # [128, 64]
    jwrest = bf(np.concatenate([jw2t_sb, jw3t_sb, jw4t], axis=1))

    # bias pack [128, 8]: jb1 (4 cols), jb2 (2), jb3 (1), jb4 (1, rows 0:64)
    biases = np.zeros((128, 8), np.float32)
    biases[:, 0:4] = f(inputs["jb1"]).reshape(4, 128).T
    biases[:, 4:6] = f(inputs["jb2"]).reshape(2, 128).T
    biases[:, 6] = f(inputs["jb3"])
    biases[0:64, 7] = f(inputs["jb4"])

    brow = zrn(np.concatenate([b_ih[0:128] + b_hh[0:128], b_ih[128:192]]))
    wgobt = np.concatenate([zrn(W_ih[:, 3:6].T), brow[None, :]])  # [4, 192]
    brow2 = np.concatenate([np.zeros(128, np.float32), b_hh[128:192]])
    whhbt = np.concatenate([zrn(W_hh.T), brow2[None, :]])         # [65, 192]

    ow1bt = np.concatenate([oW1.T, ob1[None, :]])            # [65, 4]
    w23 = oW2.T @ oW3.T                                      # [4, 3]
    b23 = ob2 @ oW3.T + ob3                                  # [3]
    ow23bt = np.zeros((33, 3), np.float32)
    ow23bt[0:4] = w23
    ow23bt[32] = b23
    # x-recurrence folded through d1:  W_ihx @ dx = (W23 @ W_ihx.T).T@d1...
    wixobt = np.zeros((33, 192), np.float32)
    wixobt[0:4] = zrn(w23 @ W_ih[:, 0:3].T)                  # [4, 192]
    wixobt[32] = zrn(W_ih[:, 0:3] @ b23)                     # [192]

    goalT = np.stack([gp_[:, 0, 3], gp_[:, 1, 3], gps])      # [3, 256]

    gpack = np.zeros((128, GP_COLS), np.float32)
    gpack[0:65, 0:192] = whhbt
    gpack[0:65, 192:196] = ow1bt
    gpack[0:33, 196:388] = wixobt
    gpack[0:33, 388:391] = ow23bt
    gpack[0:64, 391:583] = -whhbt[0:64]
    gpack[0:64, 583:587] = -ow1bt[0:64]

    in_maps = []
    for i in range(N_CORES):
        sl = slice(i * B_SH, (i + 1) * B_SH)
        go = np.concatenate(
            [goalT[:, sl], np.ones((1, B_SH), np.float32)])  # [4, 32]
        epack = np.zeros((128, EP_COLS), np.float32)
        epack[0:4, 0:192] = wgobt
        epack[0:4, 192:192 + B_SH] = go
        epack[0:128, 192 + B_SH:EP_COLS] = biases
        in_maps.append(dict(
            z=np.ascontiguousarray(z[sl].reshape(ROWS, S)),
            w1p=jw1t_sb, wrest=jwrest, epack=epack, gpack=gpack,
        ))
    return in_maps


def unshard_out(results: list[dict]) -> np.ndarray:
    # per-core out [24, 32]: row 3t+c, col b  ->  [32, 8, 3]
    parts = [r["out"].reshape(T, 3, B_SH).transpose(2, 0, 1) for r in results]
    return np.ascontiguousarray(np.concatenate(parts, axis=0), dtype=np.float32)


def kernel(**inputs) -> np.ndarray:
    from concourse.bass_utils import run_bass_kernel_spmd

    nc = _get_program()
    in_maps = make_in_maps(**inputs)
    res = run_bass_kernel_spmd(nc, in_maps, core_ids=list(range(N_CORES)))
    return unshard_out(res.results)
